# revision 17
# baseline (speedup 1.0000x reference)
"""Trainium2 Bass kernel for MaterialsGraphSAGE (4-layer GraphSAGE + pooling).

Strategy (8 NeuronCores, one chip):
- Node space padded to 50176 = 8 x 6272; core c owns nodes [6272c, 6272(c+1)).
- Edges are owned by their dst core, grouped per 128-node dst block, split by
  src half (dma_gather idx is int16, so the h table is addressed as two 25088
  row halves), padded to 128-edge tiles. Tile counts per (block, half) are
  normalized to the max across cores so the SPMD program structure is
  core-independent; only the idx / S data differs per core.
- The scatter-mean matrices S[e, dst] (one-hot scaled by 1/deg[dst]) are
  precomputed on the host from edge_index and streamed from DRAM per layer,
  so no per-layer one-hot construction happens on device.
- Per layer: dma_gather preps (prepare_only + trigger_dma) fetch h[src] rows;
  the scatter-mean is a matmul against the streamed S accumulated in PSUM
  (transposed: meanT[f, n]); the dense SAGE update + BN runs in transposed
  layout so per-channel affine ops are per-partition. Each core's new h slice
  is written to DRAM and AllGathered into a pair-shared full table.
- Final layer accumulates graph pooling (one-hot over graph ids) + counts;
  contributions ride a small AllReduce; every core computes the tiny final
  MLP; core 0's output is returned.
"""

import sys

for _p in ("/opt/trn_rl_repo",):
    if _p not in sys.path:
        sys.path.insert(0, _p)

import ml_dtypes
import numpy as np

import concourse.bacc as bacc
import concourse.mybir as mybir
import concourse.tile as tile
from concourse.bass_utils import run_bass_kernel_spmd
from concourse.vector_clock import ScopedClock

F32 = mybir.dt.float32
BF16 = mybir.dt.bfloat16
I16 = mybir.dt.int16
FP8 = mybir.dt.float8e4

P = 128
NCORES = 8
NN = 50000
NG = 256
SLICE = 6272
PADN = SLICE * NCORES      # 50176
HALF = PADN // 2           # 25088
NB = SLICE // P            # 49 blocks per core
LO_B = 31                  # blocks in the lo table half (8*LO_B*128 < 32768)
LO_R = LO_B * P            # 3968 rows per core in lo half
HI_R = SLICE - LO_R        # 2304 rows per core in hi half
NL = 4                     # SAGE layers
H = 128
NODE_F = 64
CH_TILES = 8               # gather tiles per dma_gather call
NEG_SLOPE = 0.01
BN_EPS = 1e-5


# ---------------------------------------------------------------------------
# walrus in this container rejects >1 sync wait per instruction; split them.
def _patch_tile_drain():
    def _drain_and_barrier(self, tick_clock, wait_clock):
        drain_inst = self.nc.sync.drain()
        wait_clock.add_sem_waits(
            drain_inst.ins, ScopedClock({None: tick_clock.global_clock})
        )
        si = drain_inst.ins.sync_info
        waits = list(si.on_wait) if si is not None else []
        if len(waits) > 1:
            drain_inst.ins.sync_info = mybir.SyncInfo(
                on_wait=[waits[0]], on_update=list(si.on_update)
            )
            for w in waits[1:]:
                extra = self.nc.sync.drain()
                extra.ins.sync_info = mybir.SyncInfo(on_wait=[w], on_update=[])
        self.nc.all_engine_barrier()
        assert self.sems is not None
        popped = self.nc._tile_sem_poison_stack.pop()
        assert popped is self._sem_poison
        self.nc.clear_and_free_semaphores(list(self.sems.allocated().values()))
        self.nc.all_engine_barrier()

    tile.TileContext._drain_and_barrier = _drain_and_barrier


_patch_tile_drain()


def _legalize_sync_waits(nc, max_waits=1):
    for fn in nc.m.functions:
        for bb in fn.blocks:
            out = []
            changed = False
            for ins in bb.instructions:
                si = ins.sync_info
                if si is not None and len(si.on_wait) > max_waits:
                    waits = list(si.on_wait)
                    for w in waits[:-max_waits]:
                        nop = mybir.InstNoOp(
                            name=f"WSPLIT-{nc.next_id()}", ins=[], outs=[]
                        )
                        nop.engine = ins.engine
                        nop.sync_info = mybir.SyncInfo(on_wait=[w], on_update=[])
                        out.append(nop)
                    ins.sync_info = mybir.SyncInfo(
                        on_wait=waits[-max_waits:], on_update=list(si.on_update)
                    )
                    changed = True
                out.append(ins)
            if changed:
                bb.instructions = out


# ---------------------------------------------------------------------------
def _wrap_idx(flat):
    """int16 row indices -> dma_gather idx buffer [128, n/16] (wrapped in 16
    partitions, replicated across the 8 Q7 core groups)."""
    n = flat.shape[0]
    assert n % 16 == 0
    buf = np.zeros((P, n // 16), np.int16)
    j = np.arange(n)
    for k in range(8):
        buf[16 * k + (j % 16), j // 16] = flat
    return buf


def _prepare(src, dst, batch_gid):
    """Group edges per core / dst block / src half; normalize tile counts
    across cores so all cores share one program structure. Precompute the
    scatter matrices S[e, dst] = 1/deg[dst] (one tile per 128-edge group)."""
    deg = np.bincount(dst, minlength=NN).astype(np.float32)
    inv_deg = np.ones(PADN, np.float32)
    inv_deg[:NN] = 1.0 / np.maximum(deg, 1.0)

    per_core = []
    for c in range(NCORES):
        base = c * SLICE
        m = (dst >= base) & (dst < base + SLICE)
        s = src[m]
        d = dst[m]
        blk = (d - base) >> 7
        sc = s // SLICE
        sr = s % SLICE
        half = (sr >= LO_R).astype(np.int64)
        key = blk * 2 + half
        order = np.argsort(key, kind="stable")
        s, d, key = s[order], d[order], key[order]
        sc, sr = sc[order], sr[order]
        bounds = np.searchsorted(key, np.arange(2 * NB + 1))
        cells = {}
        for b in range(NB):
            for h in (0, 1):
                lo, hi = bounds[2 * b + h], bounds[2 * b + h + 1]
                if hi > lo:
                    if h == 0:
                        sl = (sc[lo:hi] * LO_R + sr[lo:hi]).astype(np.int16)
                    else:
                        sl = (sc[lo:hi] * HI_R
                              + (sr[lo:hi] - LO_R)).astype(np.int16)
                    doff = (d[lo:hi] - base - b * P).astype(np.int64)
                    cells[(b, h)] = (sl, doff)
        per_core.append(cells)

    # normalized tile counts
    NT = np.zeros((NB, 2), np.int64)
    for b in range(NB):
        for h in (0, 1):
            n = max((len(per_core[c].get((b, h), ((), ()))[0])
                     for c in range(NCORES)), default=0)
            NT[b, h] = -(-n // P)
        if NT[b].sum() == 0:
            NT[b, 0] = 1

    nt_lo = int(NT[:, 0].sum())
    nt_hi = int(NT[:, 1].sum())
    nt_tot = nt_lo + nt_hi

    # shared structure: stream positions and block refs
    pos = {0: 0, 1: 0}
    block_refs = [[] for _ in range(NB)]
    tile_pos = {}              # (b,h,t) -> (stream, stream_pos, gidx)
    for b in range(NB):
        for h in (0, 1):
            for t in range(int(NT[b, h])):
                p_ = pos[h]
                g = p_ if h == 0 else nt_lo + p_
                block_refs[b].append((h, p_ // CH_TILES, p_ % CH_TILES, g))
                tile_pos[(b, h, t)] = (h, p_, g)
                pos[h] += 1

    # chunk sizes per stream (last may be partial)
    chunks = {}
    for h, nt in ((0, nt_lo), (1, nt_hi)):
        chunks[h] = [min(CH_TILES, nt - c0) for c0 in range(0, nt, CH_TILES)]

    # per-core data buffers
    core_data = []
    rowsel = np.arange(P)
    for c in range(NCORES):
        flat = {0: np.zeros(nt_lo * P, np.int16),
                1: np.zeros(nt_hi * P, np.int16)}
        S = np.zeros((P, nt_tot, P), np.float32)
        for b in range(NB):
            for h in (0, 1):
                sl, doff = per_core[c].get(
                    (b, h), (np.zeros(0, np.int16), np.zeros(0, np.int64)))
                n = len(sl)
                for t in range(int(NT[b, h])):
                    _, p_, g = tile_pos[(b, h, t)]
                    seg_s = sl[t * P:(t + 1) * P]
                    seg_d = doff[t * P:(t + 1) * P]
                    flat[h][p_ * P:p_ * P + len(seg_s)] = seg_s
                    if len(seg_d):
                        S[rowsel[:len(seg_d)], g, seg_d] = 1.0
        base_c = c * SLICE
        invb = np.broadcast_to(inv_deg[base_c:base_c + SLICE],
                               (P, SLICE)).astype(ml_dtypes.bfloat16)
        core_data.append(dict(
            idx_lo=_wrap_idx(flat[0]) if nt_lo else np.zeros((P, 8), np.int16),
            idx_hi=_wrap_idx(flat[1]) if nt_hi else np.zeros((P, 8), np.int16),
            stab=S.reshape(P, nt_tot * P).astype(ml_dtypes.float8_e4m3),
            invb=np.ascontiguousarray(invb),
        ))

    # pooling one-hot [128, NB*NG] (pad nodes -> all-zero rows)
    ghots = []
    garange = np.arange(NG, dtype=np.float32)
    for c in range(NCORES):
        base = c * SLICE
        col = np.full(SLICE, -1.0, np.float32)
        npad = min(max(NN - base, 0), SLICE)
        if npad > 0:
            col[:npad] = batch_gid[base:base + npad]
        gid_pb = col.reshape(NB, P).T          # [P, NB]
        ghot = (gid_pb[:, :, None] == garange[None, None, :])
        ghots.append(np.ascontiguousarray(
            ghot.reshape(P, NB * NG)).astype(ml_dtypes.float8_e4m3))

    return dict(nt_lo=nt_lo, nt_hi=nt_hi, chunks=chunks,
                block_refs=block_refs, core_data=core_data, ghots=ghots)


def kernel(x, edge_index, u, batch, W_emb, b_emb, W_l, b_l, W_r, gamma, beta,
           W_g, b_g, W_f1, b_f1, W_f2, b_f2):
    x = np.asarray(x, np.float32)
    edge_index = np.asarray(edge_index)
    u = np.asarray(u, np.float32)
    batch = np.asarray(batch)

    src = edge_index[0].astype(np.int64)
    dst = edge_index[1].astype(np.int64)
    prep = _prepare(src, dst, batch.astype(np.float32))

    nt_lo, nt_hi = prep["nt_lo"], prep["nt_hi"]
    nt_tot = nt_lo + nt_hi
    chunks = prep["chunks"]
    block_refs = prep["block_refs"]

    xT = np.zeros((NODE_F, PADN), ml_dtypes.bfloat16)
    xT[:, :NN] = x.T.astype(ml_dtypes.bfloat16)

    inv_std = np.float32(1.0 / np.sqrt(1.0 + BN_EPS))
    gscale = np.asarray(gamma, np.float32) * inv_std  # [L, H]
    beta_np = np.asarray(beta, np.float32)

    ident_np = np.eye(P, dtype=ml_dtypes.bfloat16)
    ones_np = np.ones((P, 1), np.float32)

    # ------------------------------------------------------------------
    nc = bacc.Bacc(None, num_swdge_queues=4, dynamic_dma_scratch_size=32768)

    def din(name, shape, dtype=F32):
        return nc.dram_tensor(name, shape, dtype, kind="ExternalInput")

    xT_in = din("xT", [NODE_F, SLICE], BF16)
    idx_lo_in = din("idx_lo", [P, max(nt_lo, 1) * 8], I16)
    idx_hi_in = din("idx_hi", [P, max(nt_hi, 1) * 8], I16)
    stab_in = din("stab", [P, nt_tot * P], FP8)
    invb_in = din("invb", [P, SLICE], BF16)
    ghot_in = din("ghot", [P, NB * NG], FP8)
    ident_in = din("ident", [P, P], BF16)
    ones_in = din("ones", [P, 1])
    wemb_in = din("wemb", [NODE_F, H], BF16)
    bemb_in = din("bemb", [P, 1])
    wl_in = din("wl", [H, NL * H], BF16)
    wr_in = din("wr", [H, NL * H], BF16)
    gsbl_in = din("gsbl", [P, NL])
    gs_in = din("gs", [P, NL])
    bt_in = din("bt", [P, NL])
    uT_in = din("uT", [16, NG])
    wg_in = din("wg", [16, H])
    bg_in = din("bg", [P, 1])
    wf1_in = din("wf1", [2 * H, H])
    bf1_in = din("bf1", [P, 1])
    wf2_in = din("wf2", [H, 2])
    bf2_in = din("bf2", [2, 1])
    y_out = nc.dram_tensor("y", [2, NG], F32, kind="ExternalOutput")

    RG = [list(range(NCORES))]
    AluOp = mybir.AluOpType
    Act = mybir.ActivationFunctionType

    n_sch = -(-nt_tot // CH_TILES)   # S stream chunks
    sch_sizes = [min(CH_TILES, nt_tot - c0)
                 for c0 in range(0, nt_tot, CH_TILES)]

    with tile.TileContext(nc) as tc:
        with (
            tc.tile_pool(name="dram", bufs=1, space="DRAM") as dram,
            tc.tile_pool(name="meta", bufs=1) as meta,
            tc.tile_pool(name="hbuf", bufs=1) as hbuf,
            tc.tile_pool(name="glo", bufs=4) as glo,
            tc.tile_pool(name="ghi", bufs=4) as ghi,
            tc.tile_pool(name="oh", bufs=2) as ohp,
            tc.tile_pool(name="mean", bufs=3) as meanp,
            tc.tile_pool(name="pre", bufs=3) as prep_,
            tc.tile_pool(name="stg", bufs=3) as stgp,
            tc.tile_pool(name="ps_scat", bufs=2, space="PSUM") as ps_scat,
            tc.tile_pool(name="ps_dense", bufs=2, space="PSUM") as ps_dense,
            tc.tile_pool(name="ps_tr", bufs=2, space="PSUM") as ps_tr,
            tc.tile_pool(name="ps_pool", bufs=1, space="PSUM") as ps_pool,
            tc.tile_pool(name="ps_cnt", bufs=1, space="PSUM") as ps_cnt,
            tc.tile_pool(name="small", bufs=1) as small,
        ):
            # ---- constants & metadata
            idx_lo = meta.tile([P, max(nt_lo, 1) * 8], I16)
            idx_hi = meta.tile([P, max(nt_hi, 1) * 8], I16)
            stab_t = meta.tile([P, nt_tot * P], FP8)
            invb_t = meta.tile([P, SLICE], BF16)
            ghot_t = meta.tile([P, NB * NG], FP8)
            ident_t = meta.tile([P, P], BF16)
            ones_t = meta.tile([P, 1], F32)
            ones_bf = meta.tile([P, 1], BF16)
            xT_t = meta.tile([NODE_F, SLICE], BF16)
            wemb_t = meta.tile([NODE_F, H], BF16)
            bemb_t = meta.tile([P, 1], F32)
            wl_t = meta.tile([H, NL * H], BF16)
            wr_t = meta.tile([H, NL * H], BF16)
            gsbl_t = meta.tile([P, NL], F32)
            gs_t = meta.tile([P, NL], F32)
            bt_t = meta.tile([P, NL], F32)
            uT_t = meta.tile([16, NG], F32)
            wg_t = meta.tile([16, H], F32)
            bg_t = meta.tile([P, 1], F32)
            wf1a_t = meta.tile([H, H], F32)
            wf1b_t = meta.tile([H, H], F32)
            bf1_t = meta.tile([P, 1], F32)
            wf2_t = meta.tile([H, 2], F32)
            bf2_t = meta.tile([2, 1], F32)
            for t_, i_ in (
                (idx_lo, idx_lo_in), (idx_hi, idx_hi_in),
                (stab_t, stab_in), (invb_t, invb_in),
                (ghot_t, ghot_in),
                (ident_t, ident_in), (ones_t, ones_in), (xT_t, xT_in),
                (wemb_t, wemb_in), (bemb_t, bemb_in), (wl_t, wl_in),
                (wr_t, wr_in), (gsbl_t, gsbl_in), (gs_t, gs_in), (bt_t, bt_in),
                (uT_t, uT_in), (wg_t, wg_in), (bg_t, bg_in),
                (wf1a_t, wf1_in[:H, :]), (wf1b_t, wf1_in[H:, :]),
                (bf1_t, bf1_in), (wf2_t, wf2_in), (bf2_t, bf2_in),
            ):
                nc.sync.dma_start(t_[:], i_[:])

            nc.vector.tensor_copy(ones_bf[:], ones_t[:])
            hT_a = hbuf.tile([P, SLICE], BF16, name="hT_a")
            hT_b = hbuf.tile([P, SLICE], BF16, name="hT_b")

            slices_lo = [dram.tile([LO_R, H], BF16, name=f"slicelo{i}")
                         for i in range(NL)]
            slices_hi = [dram.tile([HI_R, H], BF16, name=f"slicehi{i}")
                         for i in range(NL)]
            tab_lo = [dram.tile([NCORES * LO_R, H], BF16, addr_space="Shared",
                                name=f"tablo{i}") for i in range(NL)]
            tab_hi = [dram.tile([NCORES * HI_R, H], BF16, addr_space="Shared",
                                name=f"tabhi{i}") for i in range(NL)]
            payload = dram.tile([P + 1, NG], F32, name="payload")

            # ---- embed ----------------------------------------------------
            _sc_embed = nc.enter_named_scope("embed", False)
            for b in range(NB):
                hp = ps_dense.tile([P, P], F32, tag="d")
                nc.tensor.matmul(out=hp[:], lhsT=wemb_t[:],
                                 rhs=xT_t[:, b * P:(b + 1) * P],
                                 start=True, stop=True)
                nc.scalar.activation(hT_a[:, b * P:(b + 1) * P], hp[:],
                                     Act.Lrelu, bias=bemb_t[:], scale=1.0,
                                     alpha=NEG_SLOPE)
                tp = ps_tr.tile([P, P], BF16, tag="t")
                nc.tensor.transpose(out=tp[:], in_=hT_a[:, b * P:(b + 1) * P],
                                    identity=ident_t[:])
                stg = stgp.tile([P, P], BF16, tag="st")
                nc.vector.tensor_copy(stg[:], tp[:])
                if b < LO_B:
                    nc.sync.dma_start(slices_lo[0][b * P:(b + 1) * P, :],
                                      stg[:])
                else:
                    nc.sync.dma_start(
                        slices_hi[0][(b - LO_B) * P:(b - LO_B + 1) * P, :],
                        stg[:])
                if b == LO_B - 1:
                    nc.gpsimd.collective_compute(
                        "AllGather", AluOp.bypass, replica_groups=RG,
                        ins=[slices_lo[0][:]], outs=[tab_lo[0][:]],
                    )

            nc.gpsimd.collective_compute(
                "AllGather", AluOp.bypass, replica_groups=RG,
                ins=[slices_hi[0][:]], outs=[tab_hi[0][:]],
            )
            nc.leave_named_scope("embed", _sc_embed[0], False)

            # ---- SAGE layers ---------------------------------------------
            hT_prev, hT_new = hT_a, hT_b
            pool_ps = ps_pool.tile([P, NG], F32, tag="pp")
            gcnt_ps = ps_cnt.tile([1, NG], F32, tag="c")

            for li in range(NL):
                _sc_l = nc.enter_named_scope(f"layer{li}", False)

                # gather preps + triggers (interleave lo/hi)
                sched = []
                for h, idx_t_, pool_h in ((0, idx_lo, glo), (1, idx_hi, ghi)):
                    c0 = 0
                    for ntc in chunks[h]:
                        sched.append((h, idx_t_, pool_h, ntc, c0))
                        c0 += ntc
                lo_s = [e for e in sched if e[0] == 0]
                hi_s = [e for e in sched if e[0] == 1]
                inter = []
                i = j = 0
                while i < len(lo_s) or j < len(hi_s):
                    if i < len(lo_s):
                        inter.append(lo_s[i]); i += 1
                    if j < len(hi_s):
                        inter.append(hi_s[j]); j += 1
                chunk_tiles = {0: [], 1: []}
                qrr = 0
                for h, idx_t_, pool_h, ntc, c0 in inter:
                    g = pool_h.tile([P, ntc, P], BF16, tag=f"g{h}")
                    nidx = ntc * P
                    fs = nidx // 16
                    f0 = c0 * P // 16
                    nc.gpsimd.dma_gather(
                        out_ap=g[:],
                        in_ap=(tab_lo[li] if h == 0 else tab_hi[li])[:],
                        idxs_ap=idx_t_[:, f0:f0 + fs],
                        num_idxs=nidx, num_idxs_reg=nidx, elem_size=H,
                        queue_num=qrr % 4, single_packet=False,
                    )
                    qrr += 1
                    chunk_tiles[h].append(g)

                for b in range(NB):
                    sp = ps_scat.tile([P, P], F32, tag="sc")
                    refs = block_refs[b]
                    for i_r, (st, ch, slot, g) in enumerate(refs):
                        xs = chunk_tiles[st][ch][:, slot, :]
                        sref = stab_t[:, g * P:(g + 1) * P]
                        nc.tensor.matmul(out=sp[:], lhsT=xs, rhs=sref,
                                         start=(i_r == 0),
                                         stop=(i_r == len(refs) - 1))
                    mt = meanp.tile([P, P], BF16, tag="m")
                    nc.vector.tensor_tensor(
                        out=mt[:], in0=sp[:],
                        in1=invb_t[:, b * P:(b + 1) * P], op=AluOp.mult)
                    hp = ps_dense.tile([P, P], F32, tag="d")
                    nc.tensor.matmul(out=hp[:],
                                     lhsT=wl_t[:, li * H:(li + 1) * H],
                                     rhs=mt[:], start=True, stop=False)
                    nc.tensor.matmul(out=hp[:],
                                     lhsT=wr_t[:, li * H:(li + 1) * H],
                                     rhs=hT_prev[:, b * P:(b + 1) * P],
                                     start=False, stop=True)
                    pre = prep_.tile([P, P], F32, tag="p")
                    nc.scalar.activation(pre[:], hp[:], Act.Lrelu,
                                         bias=gsbl_t[:, li:li + 1],
                                         scale=gs_t[:, li:li + 1],
                                         alpha=NEG_SLOPE)
                    nc.vector.tensor_scalar(
                        out=hT_new[:, b * P:(b + 1) * P], in0=pre[:],
                        scalar1=bt_t[:, li:li + 1], scalar2=None,
                        op0=AluOp.add)
                    tp = ps_tr.tile([P, P], BF16, tag="t")
                    nc.tensor.transpose(out=tp[:],
                                        in_=hT_new[:, b * P:(b + 1) * P],
                                        identity=ident_t[:])
                    stg = stgp.tile([P, P], BF16, tag="st")
                    nc.vector.tensor_copy(stg[:], tp[:])
                    if li < NL - 1:
                        if b < LO_B:
                            nc.sync.dma_start(
                                slices_lo[li + 1][b * P:(b + 1) * P, :],
                                stg[:])
                        else:
                            nc.sync.dma_start(
                                slices_hi[li + 1][
                                    (b - LO_B) * P:(b - LO_B + 1) * P, :],
                                stg[:])
                        if b == LO_B - 1:
                            nc.gpsimd.collective_compute(
                                "AllGather", AluOp.bypass, replica_groups=RG,
                                ins=[slices_lo[li + 1][:]],
                                outs=[tab_lo[li + 1][:]],
                            )
                    else:
                        gb = ghot_t[:, b * NG:(b + 1) * NG]
                        nc.tensor.matmul(out=pool_ps[:], lhsT=stg[:], rhs=gb,
                                         start=(b == 0), stop=(b == NB - 1))
                        nc.tensor.matmul(out=gcnt_ps[:], lhsT=ones_bf[:],
                                         rhs=gb,
                                         start=(b == 0), stop=(b == NB - 1))

                if li < NL - 1:
                    nc.gpsimd.collective_compute(
                        "AllGather", AluOp.bypass, replica_groups=RG,
                        ins=[slices_hi[li + 1][:]], outs=[tab_hi[li + 1][:]],
                    )
                hT_prev, hT_new = hT_new, hT_prev
                nc.leave_named_scope(f"layer{li}", _sc_l[0], False)

            # ---- pooling epilogue ----------------------------------------
            _sc_e = nc.enter_named_scope("epilogue", False)
            poolT = small.tile([P, NG], F32, tag="poolT")
            nc.vector.tensor_copy(poolT[:], pool_ps[:])
            gcrow = small.tile([1, NG], F32, tag="gcrow")
            nc.vector.tensor_copy(gcrow[:], gcnt_ps[:])
            nc.sync.dma_start(payload[:P, :], poolT[:])
            nc.sync.dma_start(payload[P:P + 1, :], gcrow[:])
            nc.gpsimd.collective_compute(
                "AllReduce", AluOp.add, replica_groups=RG,
                ins=[payload[:]], outs=[payload[:]],
            )
            pool_acc = small.tile([P, NG], F32, tag="pacc")
            gc_acc = small.tile([1, NG], F32, tag="gacc")
            nc.sync.dma_start(pool_acc[:], payload[:P, :])
            nc.sync.dma_start(gc_acc[:], payload[P:P + 1, :])
            nc.vector.tensor_scalar(out=gc_acc[:], in0=gc_acc[:], scalar1=1.0,
                                    scalar2=None, op0=AluOp.max)
            nc.vector.reciprocal(gc_acc[:], gc_acc[:])
            invg_row_d = dram.tile([1, NG], F32, name="invg_row_d")
            nc.sync.dma_start(invg_row_d[:], gc_acc[:])
            invg_bc = small.tile([P, NG], F32, tag="invgbc")
            nc.sync.dma_start(invg_bc[:],
                              invg_row_d[:1, :].to_broadcast((P, NG)))
            nc.vector.tensor_tensor(out=pool_acc[:], in0=pool_acc[:],
                                    in1=invg_bc[:], op=AluOp.mult)

            ug_ps = ps_dense.tile([P, NG], F32, tag="d")
            nc.tensor.matmul(out=ug_ps[:], lhsT=wg_t[:], rhs=uT_t[:],
                             start=True, stop=True)
            ugT = small.tile([P, NG], F32, tag="ugT")
            nc.scalar.activation(ugT[:], ug_ps[:], Act.Lrelu, bias=bg_t[:],
                                 scale=1.0, alpha=NEG_SLOPE)

            hid_ps = ps_dense.tile([P, NG], F32, tag="d")
            nc.tensor.matmul(out=hid_ps[:], lhsT=wf1a_t[:],
                             rhs=pool_acc[:], start=True, stop=False)
            nc.tensor.matmul(out=hid_ps[:], lhsT=wf1b_t[:], rhs=ugT[:],
                             start=False, stop=True)
            hidT = small.tile([P, NG], F32, tag="hidT")
            nc.scalar.activation(hidT[:], hid_ps[:], Act.Lrelu, bias=bf1_t[:],
                                 scale=1.0, alpha=NEG_SLOPE)

            y_ps = ps_tr.tile([2, NG], F32, tag="t")
            nc.tensor.matmul(out=y_ps[:], lhsT=wf2_t[:], rhs=hidT[:],
                             start=True, stop=True)
            yT = small.tile([2, NG], F32, tag="yT")
            nc.vector.tensor_scalar(out=yT[:], in0=y_ps[:], scalar1=bf2_t[:],
                                    scalar2=None, op0=AluOp.add)
            nc.sync.dma_start(y_out[:], yT[:])
            nc.leave_named_scope("epilogue", _sc_e[0], False)

    nc.finalize()
    _legalize_sync_waits(nc)

    common = dict(
        ident=ident_np, ones=ones_np,
        wemb=np.asarray(W_emb, np.float32).astype(ml_dtypes.bfloat16),
        bemb=np.asarray(b_emb, np.float32).reshape(P, 1),
        wl=np.asarray(W_l, np.float32).transpose(1, 0, 2).reshape(H, NL * H).astype(ml_dtypes.bfloat16),
        wr=np.asarray(W_r, np.float32).transpose(1, 0, 2).reshape(H, NL * H).astype(ml_dtypes.bfloat16),
        gsbl=(gscale * np.asarray(b_l, np.float32)).T.copy(),
        gs=gscale.T.copy(), bt=beta_np.T.copy(),
        uT=u.T.copy(),
        wg=np.asarray(W_g, np.float32),
        bg=np.asarray(b_g, np.float32).reshape(P, 1),
        wf1=np.asarray(W_f1, np.float32),
        bf1=np.asarray(b_f1, np.float32).reshape(P, 1),
        wf2=np.asarray(W_f2, np.float32),
        bf2=np.asarray(b_f2, np.float32).reshape(2, 1),
    )
    in_maps = []
    for c in range(NCORES):
        cd = prep["core_data"][c]
        in_maps.append(dict(
            common,
            xT=xT[:, c * SLICE:(c + 1) * SLICE].copy(),
            idx_lo=cd["idx_lo"], idx_hi=cd["idx_hi"],
            stab=cd["stab"], invb=cd["invb"], ghot=prep["ghots"][c],
        ))

    res = run_bass_kernel_spmd(nc, in_maps, core_ids=list(range(NCORES)),
                               trace=TRACE)
    global LAST_RESULT
    LAST_RESULT = res
    return np.asarray(res.results[0]["y"]).T.astype(np.float32).copy()


TRACE = False
LAST_RESULT = None


# revision 22
# speedup vs baseline: 1.1259x; 1.1259x over previous
"""Trainium2 Bass kernel for MaterialsGraphSAGE (4-layer GraphSAGE + pooling).

Strategy (8 NeuronCores, one chip):
- Node space padded to 50176 = 8 x 6272; core c owns nodes [6272c, 6272(c+1)).
- Edges are owned by their dst core, grouped per 128-node dst block, split by
  src half (dma_gather idx is int16, so the h table is addressed as two 25088
  row halves), padded to 128-edge tiles. Tile counts per (block, half) are
  normalized to the max across cores so the SPMD program structure is
  core-independent; only the idx / S data differs per core.
- The scatter-mean matrices S[e, dst] (one-hot scaled by 1/deg[dst]) are
  precomputed on the host from edge_index and streamed from DRAM per layer,
  so no per-layer one-hot construction happens on device.
- Per layer: dma_gather preps (prepare_only + trigger_dma) fetch h[src] rows;
  the scatter-mean is a matmul against the streamed S accumulated in PSUM
  (transposed: meanT[f, n]); the dense SAGE update + BN runs in transposed
  layout so per-channel affine ops are per-partition. Each core's new h slice
  is written to DRAM and AllGathered into a pair-shared full table.
- Final layer accumulates graph pooling (one-hot over graph ids) + counts;
  contributions ride a small AllReduce; every core computes the tiny final
  MLP; core 0's output is returned.
"""

import sys

for _p in ("/opt/trn_rl_repo",):
    if _p not in sys.path:
        sys.path.insert(0, _p)

import ml_dtypes
import numpy as np

import concourse.bacc as bacc
import concourse.mybir as mybir
import concourse.tile as tile
from concourse.bass_utils import run_bass_kernel_spmd
from concourse.vector_clock import ScopedClock

F32 = mybir.dt.float32
BF16 = mybir.dt.bfloat16
I16 = mybir.dt.int16
FP8 = mybir.dt.float8e4

P = 128
NCORES = 8
NN = 50000
NG = 256
SLICE = 6272
PADN = SLICE * NCORES      # 50176
HALF = PADN // 2           # 25088
NB = SLICE // P            # 49 blocks per core
NL = 4                     # SAGE layers
H = 128
NODE_F = 64
CH_TILES = 8               # gather tiles per dma_gather call
NEG_SLOPE = 0.01
BN_EPS = 1e-5


# ---------------------------------------------------------------------------
# walrus in this container rejects >1 sync wait per instruction; split them.
def _patch_tile_drain():
    def _drain_and_barrier(self, tick_clock, wait_clock):
        drain_inst = self.nc.sync.drain()
        wait_clock.add_sem_waits(
            drain_inst.ins, ScopedClock({None: tick_clock.global_clock})
        )
        si = drain_inst.ins.sync_info
        waits = list(si.on_wait) if si is not None else []
        if len(waits) > 1:
            drain_inst.ins.sync_info = mybir.SyncInfo(
                on_wait=[waits[0]], on_update=list(si.on_update)
            )
            for w in waits[1:]:
                extra = self.nc.sync.drain()
                extra.ins.sync_info = mybir.SyncInfo(on_wait=[w], on_update=[])
        self.nc.all_engine_barrier()
        assert self.sems is not None
        popped = self.nc._tile_sem_poison_stack.pop()
        assert popped is self._sem_poison
        self.nc.clear_and_free_semaphores(list(self.sems.allocated().values()))
        self.nc.all_engine_barrier()

    tile.TileContext._drain_and_barrier = _drain_and_barrier


_patch_tile_drain()


def _legalize_sync_waits(nc, max_waits=1):
    for fn in nc.m.functions:
        for bb in fn.blocks:
            out = []
            changed = False
            for ins in bb.instructions:
                si = ins.sync_info
                if si is not None and len(si.on_wait) > max_waits:
                    waits = list(si.on_wait)
                    for w in waits[:-max_waits]:
                        nop = mybir.InstNoOp(
                            name=f"WSPLIT-{nc.next_id()}", ins=[], outs=[]
                        )
                        nop.engine = ins.engine
                        nop.sync_info = mybir.SyncInfo(on_wait=[w], on_update=[])
                        out.append(nop)
                    ins.sync_info = mybir.SyncInfo(
                        on_wait=waits[-max_waits:], on_update=list(si.on_update)
                    )
                    changed = True
                out.append(ins)
            if changed:
                bb.instructions = out


# ---------------------------------------------------------------------------
def _wrap_idx(flat):
    """int16 row indices -> dma_gather idx buffer [128, n/16] (wrapped in 16
    partitions, replicated across the 8 Q7 core groups)."""
    n = flat.shape[0]
    assert n % 16 == 0
    buf = np.zeros((P, n // 16), np.int16)
    j = np.arange(n)
    for k in range(8):
        buf[16 * k + (j % 16), j // 16] = flat
    return buf


def _prepare(src, dst, batch_gid):
    """Group edges per core / dst block / src half; normalize tile counts
    across cores so all cores share one program structure. Precompute the
    scatter matrices S[e, dst] = 1/deg[dst] (one tile per 128-edge group)."""
    deg = np.bincount(dst, minlength=NN).astype(np.float32)
    inv_deg = np.ones(PADN, np.float32)
    inv_deg[:NN] = 1.0 / np.maximum(deg, 1.0)

    per_core = []
    for c in range(NCORES):
        base = c * SLICE
        m = (dst >= base) & (dst < base + SLICE)
        s = src[m]
        d = dst[m]
        blk = (d - base) >> 7
        half = (s >= HALF).astype(np.int64)
        key = blk * 2 + half
        order = np.argsort(key, kind="stable")
        s, d, key = s[order], d[order], key[order]
        bounds = np.searchsorted(key, np.arange(2 * NB + 1))
        cells = {}
        for b in range(NB):
            for h in (0, 1):
                lo, hi = bounds[2 * b + h], bounds[2 * b + h + 1]
                if hi > lo:
                    sl = (s[lo:hi] - (HALF if h else 0)).astype(np.int16)
                    doff = (d[lo:hi] - base - b * P).astype(np.int64)
                    cells[(b, h)] = (sl, doff)
        per_core.append(cells)

    # normalized tile counts
    NT = np.zeros((NB, 2), np.int64)
    for b in range(NB):
        for h in (0, 1):
            n = max((len(per_core[c].get((b, h), ((), ()))[0])
                     for c in range(NCORES)), default=0)
            NT[b, h] = -(-n // P)
        if NT[b].sum() == 0:
            NT[b, 0] = 1

    nt_lo = int(NT[:, 0].sum())
    nt_hi = int(NT[:, 1].sum())
    nt_tot = nt_lo + nt_hi

    # shared structure: stream positions and block refs
    pos = {0: 0, 1: 0}
    block_refs = [[] for _ in range(NB)]
    tile_pos = {}              # (b,h,t) -> (stream, stream_pos, gidx)
    for b in range(NB):
        for h in (0, 1):
            for t in range(int(NT[b, h])):
                p_ = pos[h]
                g = p_ if h == 0 else nt_lo + p_
                block_refs[b].append((h, p_ // CH_TILES, p_ % CH_TILES, g))
                tile_pos[(b, h, t)] = (h, p_, g)
                pos[h] += 1

    # chunk sizes per stream (last may be partial)
    chunks = {}
    for h, nt in ((0, nt_lo), (1, nt_hi)):
        chunks[h] = [min(CH_TILES, nt - c0) for c0 in range(0, nt, CH_TILES)]

    # per-core data buffers
    core_data = []
    rowsel = np.arange(P)
    for c in range(NCORES):
        flat = {0: np.zeros(nt_lo * P, np.int16),
                1: np.zeros(nt_hi * P, np.int16)}
        S = np.zeros((P, nt_tot, P), np.float32)
        for b in range(NB):
            for h in (0, 1):
                sl, doff = per_core[c].get(
                    (b, h), (np.zeros(0, np.int16), np.zeros(0, np.int64)))
                n = len(sl)
                for t in range(int(NT[b, h])):
                    _, p_, g = tile_pos[(b, h, t)]
                    seg_s = sl[t * P:(t + 1) * P]
                    seg_d = doff[t * P:(t + 1) * P]
                    flat[h][p_ * P:p_ * P + len(seg_s)] = seg_s
                    if len(seg_d):
                        S[rowsel[:len(seg_d)], g, seg_d] = 1.0
        base_c = c * SLICE
        invb = np.broadcast_to(inv_deg[base_c:base_c + SLICE],
                               (P, SLICE)).astype(ml_dtypes.bfloat16)
        core_data.append(dict(
            idx_lo=_wrap_idx(flat[0]) if nt_lo else np.zeros((P, 8), np.int16),
            idx_hi=_wrap_idx(flat[1]) if nt_hi else np.zeros((P, 8), np.int16),
            stab=S.reshape(P, nt_tot * P).astype(ml_dtypes.float8_e4m3),
            invb=np.ascontiguousarray(invb),
        ))

    # pooling one-hot [128, NB*NG] (pad nodes -> all-zero rows)
    ghots = []
    garange = np.arange(NG, dtype=np.float32)
    for c in range(NCORES):
        base = c * SLICE
        col = np.full(SLICE, -1.0, np.float32)
        npad = min(max(NN - base, 0), SLICE)
        if npad > 0:
            col[:npad] = batch_gid[base:base + npad]
        gid_pb = col.reshape(NB, P).T          # [P, NB]
        ghot = (gid_pb[:, :, None] == garange[None, None, :])
        ghots.append(np.ascontiguousarray(
            ghot.reshape(P, NB * NG)).astype(ml_dtypes.float8_e4m3))

    return dict(nt_lo=nt_lo, nt_hi=nt_hi, chunks=chunks,
                block_refs=block_refs, core_data=core_data, ghots=ghots)


def kernel(x, edge_index, u, batch, W_emb, b_emb, W_l, b_l, W_r, gamma, beta,
           W_g, b_g, W_f1, b_f1, W_f2, b_f2):
    x = np.asarray(x, np.float32)
    edge_index = np.asarray(edge_index)
    u = np.asarray(u, np.float32)
    batch = np.asarray(batch)

    src = edge_index[0].astype(np.int64)
    dst = edge_index[1].astype(np.int64)
    prep = _prepare(src, dst, batch.astype(np.float32))

    nt_lo, nt_hi = prep["nt_lo"], prep["nt_hi"]
    nt_tot = nt_lo + nt_hi
    chunks = prep["chunks"]
    block_refs = prep["block_refs"]

    xT = np.zeros((NODE_F, PADN), ml_dtypes.bfloat16)
    xT[:, :NN] = x.T.astype(ml_dtypes.bfloat16)

    inv_std = np.float32(1.0 / np.sqrt(1.0 + BN_EPS))
    gscale = np.asarray(gamma, np.float32) * inv_std  # [L, H]
    beta_np = np.asarray(beta, np.float32)

    ident_np = np.eye(P, dtype=ml_dtypes.bfloat16)
    ones_np = np.ones((P, 1), np.float32)

    # ------------------------------------------------------------------
    nc = bacc.Bacc(None, num_swdge_queues=4, dynamic_dma_scratch_size=32768)

    def din(name, shape, dtype=F32):
        return nc.dram_tensor(name, shape, dtype, kind="ExternalInput")

    xT_in = din("xT", [NODE_F, SLICE], BF16)
    idx_lo_in = din("idx_lo", [P, max(nt_lo, 1) * 8], I16)
    idx_hi_in = din("idx_hi", [P, max(nt_hi, 1) * 8], I16)
    stab_in = din("stab", [P, nt_tot * P], FP8)
    invb_in = din("invb", [P, SLICE], BF16)
    ghot_in = din("ghot", [P, NB * NG], FP8)
    ident_in = din("ident", [P, P], BF16)
    ones_in = din("ones", [P, 1])
    wemb_in = din("wemb", [NODE_F, H], BF16)
    bemb_in = din("bemb", [P, 1])
    wl_in = din("wl", [H, NL * H], BF16)
    wr_in = din("wr", [H, NL * H], BF16)
    gsbl_in = din("gsbl", [P, NL])
    gs_in = din("gs", [P, NL])
    bt_in = din("bt", [P, NL])
    uT_in = din("uT", [16, NG])
    wg_in = din("wg", [16, H])
    bg_in = din("bg", [P, 1])
    wf1_in = din("wf1", [2 * H, H])
    bf1_in = din("bf1", [P, 1])
    wf2_in = din("wf2", [H, 2])
    bf2_in = din("bf2", [2, 1])
    y_out = nc.dram_tensor("y", [2, NG], F32, kind="ExternalOutput")

    RG = [list(range(NCORES))]
    AluOp = mybir.AluOpType
    Act = mybir.ActivationFunctionType

    n_sch = -(-nt_tot // CH_TILES)   # S stream chunks
    sch_sizes = [min(CH_TILES, nt_tot - c0)
                 for c0 in range(0, nt_tot, CH_TILES)]

    with tile.TileContext(nc) as tc:
        with (
            tc.tile_pool(name="dram", bufs=1, space="DRAM") as dram,
            tc.tile_pool(name="meta", bufs=1) as meta,
            tc.tile_pool(name="hbuf", bufs=1) as hbuf,
            tc.tile_pool(name="glo", bufs=4) as glo,
            tc.tile_pool(name="ghi", bufs=4) as ghi,
            tc.tile_pool(name="oh", bufs=2) as ohp,
            tc.tile_pool(name="mean", bufs=3) as meanp,
            tc.tile_pool(name="pre", bufs=3) as prep_,
            tc.tile_pool(name="stg", bufs=3) as stgp,
            tc.tile_pool(name="ps_scat", bufs=2, space="PSUM") as ps_scat,
            tc.tile_pool(name="ps_dense", bufs=2, space="PSUM") as ps_dense,
            tc.tile_pool(name="ps_tr", bufs=2, space="PSUM") as ps_tr,
            tc.tile_pool(name="ps_pool", bufs=1, space="PSUM") as ps_pool,
            tc.tile_pool(name="ps_cnt", bufs=1, space="PSUM") as ps_cnt,
            tc.tile_pool(name="small", bufs=1) as small,
        ):
            # ---- constants & metadata
            idx_lo = meta.tile([P, max(nt_lo, 1) * 8], I16)
            idx_hi = meta.tile([P, max(nt_hi, 1) * 8], I16)
            stab_t = meta.tile([P, nt_tot * P], FP8)
            invb_t = meta.tile([P, SLICE], BF16)
            ghot_t = meta.tile([P, NB * NG], FP8)
            ident_t = meta.tile([P, P], BF16)
            ones_t = meta.tile([P, 1], F32)
            ones_bf = meta.tile([P, 1], BF16)
            xT_t = meta.tile([NODE_F, SLICE], BF16)
            wemb_t = meta.tile([NODE_F, H], BF16)
            bemb_t = meta.tile([P, 1], F32)
            wl_t = meta.tile([H, NL * H], BF16)
            wr_t = meta.tile([H, NL * H], BF16)
            gsbl_t = meta.tile([P, NL], F32)
            gs_t = meta.tile([P, NL], F32)
            bt_t = meta.tile([P, NL], F32)
            uT_t = meta.tile([16, NG], F32)
            wg_t = meta.tile([16, H], F32)
            bg_t = meta.tile([P, 1], F32)
            wf1a_t = meta.tile([H, H], F32)
            wf1b_t = meta.tile([H, H], F32)
            bf1_t = meta.tile([P, 1], F32)
            wf2_t = meta.tile([H, 2], F32)
            bf2_t = meta.tile([2, 1], F32)
            for t_, i_ in (
                (idx_lo, idx_lo_in), (idx_hi, idx_hi_in),
                (stab_t, stab_in), (invb_t, invb_in),
                (ghot_t, ghot_in),
                (ident_t, ident_in), (ones_t, ones_in), (xT_t, xT_in),
                (wemb_t, wemb_in), (bemb_t, bemb_in), (wl_t, wl_in),
                (wr_t, wr_in), (gsbl_t, gsbl_in), (gs_t, gs_in), (bt_t, bt_in),
                (uT_t, uT_in), (wg_t, wg_in), (bg_t, bg_in),
                (wf1a_t, wf1_in[:H, :]), (wf1b_t, wf1_in[H:, :]),
                (bf1_t, bf1_in), (wf2_t, wf2_in), (bf2_t, bf2_in),
            ):
                nc.sync.dma_start(t_[:], i_[:])

            nc.vector.tensor_copy(ones_bf[:], ones_t[:])
            hT_a = hbuf.tile([P, SLICE], BF16, name="hT_a")
            hT_b = hbuf.tile([P, SLICE], BF16, name="hT_b")

            slices = [dram.tile([SLICE, H], BF16, name=f"slice{i}") for i in range(NL)]
            tables = [dram.tile([PADN, H], BF16, addr_space="Shared", name=f"table{i}")
                      for i in range(NL)]
            payload = dram.tile([P + 1, NG], F32, name="payload")

            # ---- embed ----------------------------------------------------
            _sc_embed = nc.enter_named_scope("embed", False)
            for b in range(NB):
                hp = ps_dense.tile([P, P], F32, tag="d")
                nc.tensor.matmul(out=hp[:], lhsT=wemb_t[:],
                                 rhs=xT_t[:, b * P:(b + 1) * P],
                                 start=True, stop=True)
                nc.scalar.activation(hT_a[:, b * P:(b + 1) * P], hp[:],
                                     Act.Lrelu, bias=bemb_t[:], scale=1.0,
                                     alpha=NEG_SLOPE)
                tp = ps_tr.tile([P, P], BF16, tag="t")
                nc.tensor.transpose(out=tp[:], in_=hT_a[:, b * P:(b + 1) * P],
                                    identity=ident_t[:])
                stg = stgp.tile([P, P], BF16, tag="st")
                nc.vector.tensor_copy(stg[:], tp[:])
                nc.sync.dma_start(slices[0][b * P:(b + 1) * P, :], stg[:])

            nc.gpsimd.collective_compute(
                "AllGather", AluOp.bypass, replica_groups=RG,
                ins=[slices[0][:]], outs=[tables[0][:]],
            )
            nc.leave_named_scope("embed", _sc_embed[0], False)

            # ---- SAGE layers ---------------------------------------------
            hT_prev, hT_new = hT_a, hT_b
            pool_ps = ps_pool.tile([P, NG], F32, tag="pp")
            gcnt_ps = ps_cnt.tile([1, NG], F32, tag="c")

            for li in range(NL):
                _sc_l = nc.enter_named_scope(f"layer{li}", False)
                table_prev = tables[li]

                # gather preps + triggers (interleave lo/hi)
                sched = []
                for h, idx_t_, pool_h in ((0, idx_lo, glo), (1, idx_hi, ghi)):
                    c0 = 0
                    for ntc in chunks[h]:
                        sched.append((h, idx_t_, pool_h, ntc, c0))
                        c0 += ntc
                lo_s = [e for e in sched if e[0] == 0]
                hi_s = [e for e in sched if e[0] == 1]
                inter = []
                i = j = 0
                while i < len(lo_s) or j < len(hi_s):
                    if i < len(lo_s):
                        inter.append(lo_s[i]); i += 1
                    if j < len(hi_s):
                        inter.append(hi_s[j]); j += 1
                chunk_tiles = {0: [], 1: []}
                qrr = 0
                for h, idx_t_, pool_h, ntc, c0 in inter:
                    g = pool_h.tile([P, ntc, P], BF16, tag=f"g{h}")
                    nidx = ntc * P
                    fs = nidx // 16
                    f0 = c0 * P // 16
                    nc.gpsimd.dma_gather(
                        out_ap=g[:],
                        in_ap=table_prev[h * HALF:(h + 1) * HALF, :],
                        idxs_ap=idx_t_[:, f0:f0 + fs],
                        num_idxs=nidx, num_idxs_reg=nidx, elem_size=H,
                        queue_num=qrr % 4, single_packet=True,
                    )
                    qrr += 1
                    chunk_tiles[h].append(g)

                for b in range(NB):
                    sp = ps_scat.tile([P, P], F32, tag="sc")
                    refs = block_refs[b]
                    for i_r, (st, ch, slot, g) in enumerate(refs):
                        xs = chunk_tiles[st][ch][:, slot, :]
                        sref = stab_t[:, g * P:(g + 1) * P]
                        nc.tensor.matmul(out=sp[:], lhsT=xs, rhs=sref,
                                         start=(i_r == 0),
                                         stop=(i_r == len(refs) - 1))
                    mt = meanp.tile([P, P], BF16, tag="m")
                    nc.vector.tensor_tensor(
                        out=mt[:], in0=sp[:],
                        in1=invb_t[:, b * P:(b + 1) * P], op=AluOp.mult)
                    hp = ps_dense.tile([P, P], F32, tag="d")
                    nc.tensor.matmul(out=hp[:],
                                     lhsT=wl_t[:, li * H:(li + 1) * H],
                                     rhs=mt[:], start=True, stop=False)
                    nc.tensor.matmul(out=hp[:],
                                     lhsT=wr_t[:, li * H:(li + 1) * H],
                                     rhs=hT_prev[:, b * P:(b + 1) * P],
                                     start=False, stop=True)
                    pre = prep_.tile([P, P], F32, tag="p")
                    nc.scalar.activation(pre[:], hp[:], Act.Lrelu,
                                         bias=gsbl_t[:, li:li + 1],
                                         scale=gs_t[:, li:li + 1],
                                         alpha=NEG_SLOPE)
                    nc.vector.tensor_scalar(
                        out=hT_new[:, b * P:(b + 1) * P], in0=pre[:],
                        scalar1=bt_t[:, li:li + 1], scalar2=None,
                        op0=AluOp.add)
                    tp = ps_tr.tile([P, P], BF16, tag="t")
                    nc.tensor.transpose(out=tp[:],
                                        in_=hT_new[:, b * P:(b + 1) * P],
                                        identity=ident_t[:])
                    stg = stgp.tile([P, P], BF16, tag="st")
                    nc.vector.tensor_copy(stg[:], tp[:])
                    if li < NL - 1:
                        nc.sync.dma_start(slices[li + 1][b * P:(b + 1) * P, :],
                                          stg[:])
                    else:
                        gb = ghot_t[:, b * NG:(b + 1) * NG]
                        nc.tensor.matmul(out=pool_ps[:], lhsT=stg[:], rhs=gb,
                                         start=(b == 0), stop=(b == NB - 1))
                        nc.tensor.matmul(out=gcnt_ps[:], lhsT=ones_bf[:],
                                         rhs=gb,
                                         start=(b == 0), stop=(b == NB - 1))

                if li < NL - 1:
                    nc.gpsimd.collective_compute(
                        "AllGather", AluOp.bypass, replica_groups=RG,
                        ins=[slices[li + 1][:]], outs=[tables[li + 1][:]],
                    )
                hT_prev, hT_new = hT_new, hT_prev
                nc.leave_named_scope(f"layer{li}", _sc_l[0], False)

            # ---- pooling epilogue ----------------------------------------
            _sc_e = nc.enter_named_scope("epilogue", False)
            poolT = small.tile([P, NG], F32, tag="poolT")
            nc.vector.tensor_copy(poolT[:], pool_ps[:])
            gcrow = small.tile([1, NG], F32, tag="gcrow")
            nc.vector.tensor_copy(gcrow[:], gcnt_ps[:])
            nc.sync.dma_start(payload[:P, :], poolT[:])
            nc.sync.dma_start(payload[P:P + 1, :], gcrow[:])
            nc.gpsimd.collective_compute(
                "AllReduce", AluOp.add, replica_groups=RG,
                ins=[payload[:]], outs=[payload[:]],
            )
            pool_acc = small.tile([P, NG], F32, tag="pacc")
            gc_acc = small.tile([1, NG], F32, tag="gacc")
            nc.sync.dma_start(pool_acc[:], payload[:P, :])
            nc.sync.dma_start(gc_acc[:], payload[P:P + 1, :])
            nc.vector.tensor_scalar(out=gc_acc[:], in0=gc_acc[:], scalar1=1.0,
                                    scalar2=None, op0=AluOp.max)
            nc.vector.reciprocal(gc_acc[:], gc_acc[:])
            invg_row_d = dram.tile([1, NG], F32, name="invg_row_d")
            nc.sync.dma_start(invg_row_d[:], gc_acc[:])
            invg_bc = small.tile([P, NG], F32, tag="invgbc")
            nc.sync.dma_start(invg_bc[:],
                              invg_row_d[:1, :].to_broadcast((P, NG)))
            nc.vector.tensor_tensor(out=pool_acc[:], in0=pool_acc[:],
                                    in1=invg_bc[:], op=AluOp.mult)

            ug_ps = ps_dense.tile([P, NG], F32, tag="d")
            nc.tensor.matmul(out=ug_ps[:], lhsT=wg_t[:], rhs=uT_t[:],
                             start=True, stop=True)
            ugT = small.tile([P, NG], F32, tag="ugT")
            nc.scalar.activation(ugT[:], ug_ps[:], Act.Lrelu, bias=bg_t[:],
                                 scale=1.0, alpha=NEG_SLOPE)

            hid_ps = ps_dense.tile([P, NG], F32, tag="d")
            nc.tensor.matmul(out=hid_ps[:], lhsT=wf1a_t[:],
                             rhs=pool_acc[:], start=True, stop=False)
            nc.tensor.matmul(out=hid_ps[:], lhsT=wf1b_t[:], rhs=ugT[:],
                             start=False, stop=True)
            hidT = small.tile([P, NG], F32, tag="hidT")
            nc.scalar.activation(hidT[:], hid_ps[:], Act.Lrelu, bias=bf1_t[:],
                                 scale=1.0, alpha=NEG_SLOPE)

            y_ps = ps_tr.tile([2, NG], F32, tag="t")
            nc.tensor.matmul(out=y_ps[:], lhsT=wf2_t[:], rhs=hidT[:],
                             start=True, stop=True)
            yT = small.tile([2, NG], F32, tag="yT")
            nc.vector.tensor_scalar(out=yT[:], in0=y_ps[:], scalar1=bf2_t[:],
                                    scalar2=None, op0=AluOp.add)
            nc.sync.dma_start(y_out[:], yT[:])
            nc.leave_named_scope("epilogue", _sc_e[0], False)

    nc.finalize()
    _legalize_sync_waits(nc)

    common = dict(
        ident=ident_np, ones=ones_np,
        wemb=np.asarray(W_emb, np.float32).astype(ml_dtypes.bfloat16),
        bemb=np.asarray(b_emb, np.float32).reshape(P, 1),
        wl=np.asarray(W_l, np.float32).transpose(1, 0, 2).reshape(H, NL * H).astype(ml_dtypes.bfloat16),
        wr=np.asarray(W_r, np.float32).transpose(1, 0, 2).reshape(H, NL * H).astype(ml_dtypes.bfloat16),
        gsbl=(gscale * np.asarray(b_l, np.float32)).T.copy(),
        gs=gscale.T.copy(), bt=beta_np.T.copy(),
        uT=u.T.copy(),
        wg=np.asarray(W_g, np.float32),
        bg=np.asarray(b_g, np.float32).reshape(P, 1),
        wf1=np.asarray(W_f1, np.float32),
        bf1=np.asarray(b_f1, np.float32).reshape(P, 1),
        wf2=np.asarray(W_f2, np.float32),
        bf2=np.asarray(b_f2, np.float32).reshape(2, 1),
    )
    in_maps = []
    for c in range(NCORES):
        cd = prep["core_data"][c]
        in_maps.append(dict(
            common,
            xT=xT[:, c * SLICE:(c + 1) * SLICE].copy(),
            idx_lo=cd["idx_lo"], idx_hi=cd["idx_hi"],
            stab=cd["stab"], invb=cd["invb"], ghot=prep["ghots"][c],
        ))

    res = run_bass_kernel_spmd(nc, in_maps, core_ids=list(range(NCORES)),
                               trace=TRACE)
    global LAST_RESULT
    LAST_RESULT = res
    return np.asarray(res.results[0]["y"]).T.astype(np.float32).copy()


TRACE = False
LAST_RESULT = None


# revision 23
# speedup vs baseline: 1.1360x; 1.0090x over previous
"""Trainium2 Bass kernel for MaterialsGraphSAGE (4-layer GraphSAGE + pooling).

Strategy (8 NeuronCores, one chip):
- Node space padded to 50176 = 8 x 6272; core c owns nodes [6272c, 6272(c+1)).
- Edges are owned by their dst core, grouped per 128-node dst block, split by
  src half (dma_gather idx is int16, so the h table is addressed as two 25088
  row halves), padded to 128-edge tiles. Tile counts per (block, half) are
  normalized to the max across cores so the SPMD program structure is
  core-independent; only the idx / S data differs per core.
- The scatter-mean matrices S[e, dst] (one-hot scaled by 1/deg[dst]) are
  precomputed on the host from edge_index and streamed from DRAM per layer,
  so no per-layer one-hot construction happens on device.
- Per layer: dma_gather preps (prepare_only + trigger_dma) fetch h[src] rows;
  the scatter-mean is a matmul against the streamed S accumulated in PSUM
  (transposed: meanT[f, n]); the dense SAGE update + BN runs in transposed
  layout so per-channel affine ops are per-partition. Each core's new h slice
  is written to DRAM and AllGathered into a pair-shared full table.
- Final layer accumulates graph pooling (one-hot over graph ids) + counts;
  contributions ride a small AllReduce; every core computes the tiny final
  MLP; core 0's output is returned.
"""

import sys

for _p in ("/opt/trn_rl_repo",):
    if _p not in sys.path:
        sys.path.insert(0, _p)

import ml_dtypes
import numpy as np

import concourse.bacc as bacc
import concourse.mybir as mybir
import concourse.tile as tile
from concourse.bass_utils import run_bass_kernel_spmd
from concourse.vector_clock import ScopedClock

F32 = mybir.dt.float32
BF16 = mybir.dt.bfloat16
I16 = mybir.dt.int16
FP8 = mybir.dt.float8e4

P = 128
NCORES = 8
NN = 50000
NG = 256
SLICE = 6272
PADN = SLICE * NCORES      # 50176
HALF = PADN // 2           # 25088
NB = SLICE // P            # 49 blocks per core
NL = 4                     # SAGE layers
H = 128
NODE_F = 64
CH_TILES = 8               # gather tiles per dma_gather call
NEG_SLOPE = 0.01
BN_EPS = 1e-5


# ---------------------------------------------------------------------------
# walrus in this container rejects >1 sync wait per instruction; split them.
def _patch_tile_drain():
    def _drain_and_barrier(self, tick_clock, wait_clock):
        drain_inst = self.nc.sync.drain()
        wait_clock.add_sem_waits(
            drain_inst.ins, ScopedClock({None: tick_clock.global_clock})
        )
        si = drain_inst.ins.sync_info
        waits = list(si.on_wait) if si is not None else []
        if len(waits) > 1:
            drain_inst.ins.sync_info = mybir.SyncInfo(
                on_wait=[waits[0]], on_update=list(si.on_update)
            )
            for w in waits[1:]:
                extra = self.nc.sync.drain()
                extra.ins.sync_info = mybir.SyncInfo(on_wait=[w], on_update=[])
        self.nc.all_engine_barrier()
        assert self.sems is not None
        popped = self.nc._tile_sem_poison_stack.pop()
        assert popped is self._sem_poison
        self.nc.clear_and_free_semaphores(list(self.sems.allocated().values()))
        self.nc.all_engine_barrier()

    tile.TileContext._drain_and_barrier = _drain_and_barrier


_patch_tile_drain()


def _legalize_sync_waits(nc, max_waits=1):
    for fn in nc.m.functions:
        for bb in fn.blocks:
            out = []
            changed = False
            for ins in bb.instructions:
                si = ins.sync_info
                if si is not None and len(si.on_wait) > max_waits:
                    waits = list(si.on_wait)
                    for w in waits[:-max_waits]:
                        nop = mybir.InstNoOp(
                            name=f"WSPLIT-{nc.next_id()}", ins=[], outs=[]
                        )
                        nop.engine = ins.engine
                        nop.sync_info = mybir.SyncInfo(on_wait=[w], on_update=[])
                        out.append(nop)
                    ins.sync_info = mybir.SyncInfo(
                        on_wait=waits[-max_waits:], on_update=list(si.on_update)
                    )
                    changed = True
                out.append(ins)
            if changed:
                bb.instructions = out


# ---------------------------------------------------------------------------
def _wrap_idx(flat):
    """int16 row indices -> dma_gather idx buffer [128, n/16] (wrapped in 16
    partitions, replicated across the 8 Q7 core groups)."""
    n = flat.shape[0]
    assert n % 16 == 0
    buf = np.zeros((P, n // 16), np.int16)
    j = np.arange(n)
    for k in range(8):
        buf[16 * k + (j % 16), j // 16] = flat
    return buf


def _prepare(src, dst, batch_gid):
    """Group edges per core / dst block / src half; normalize tile counts
    across cores so all cores share one program structure. Precompute the
    scatter matrices S[e, dst] = 1/deg[dst] (one tile per 128-edge group)."""
    deg = np.bincount(dst, minlength=NN).astype(np.float32)
    inv_deg = np.ones(PADN, np.float32)
    inv_deg[:NN] = 1.0 / np.maximum(deg, 1.0)

    per_core = []
    for c in range(NCORES):
        base = c * SLICE
        m = (dst >= base) & (dst < base + SLICE)
        s = src[m]
        d = dst[m]
        blk = (d - base) >> 7
        half = (s >= HALF).astype(np.int64)
        key = blk * 2 + half
        order = np.argsort(key, kind="stable")
        s, d, key = s[order], d[order], key[order]
        bounds = np.searchsorted(key, np.arange(2 * NB + 1))
        cells = {}
        for b in range(NB):
            for h in (0, 1):
                lo, hi = bounds[2 * b + h], bounds[2 * b + h + 1]
                if hi > lo:
                    sl = (s[lo:hi] - (HALF if h else 0)).astype(np.int16)
                    doff = (d[lo:hi] - base - b * P).astype(np.int64)
                    cells[(b, h)] = (sl, doff)
        per_core.append(cells)

    # normalized tile counts
    NT = np.zeros((NB, 2), np.int64)
    for b in range(NB):
        for h in (0, 1):
            n = max((len(per_core[c].get((b, h), ((), ()))[0])
                     for c in range(NCORES)), default=0)
            NT[b, h] = -(-n // P)
        if NT[b].sum() == 0:
            NT[b, 0] = 1

    nt_lo = int(NT[:, 0].sum())
    nt_hi = int(NT[:, 1].sum())
    nt_tot = nt_lo + nt_hi

    # shared structure: stream positions and block refs
    pos = {0: 0, 1: 0}
    block_refs = [[] for _ in range(NB)]
    tile_pos = {}              # (b,h,t) -> (stream, stream_pos, gidx)
    for b in range(NB):
        for h in (0, 1):
            for t in range(int(NT[b, h])):
                p_ = pos[h]
                g = p_ if h == 0 else nt_lo + p_
                block_refs[b].append((h, p_ // CH_TILES, p_ % CH_TILES, g))
                tile_pos[(b, h, t)] = (h, p_, g)
                pos[h] += 1

    # chunk sizes per stream (last may be partial)
    chunks = {}
    for h, nt in ((0, nt_lo), (1, nt_hi)):
        chunks[h] = [min(CH_TILES, nt - c0) for c0 in range(0, nt, CH_TILES)]

    # per-core data buffers
    core_data = []
    rowsel = np.arange(P)
    for c in range(NCORES):
        flat = {0: np.zeros(nt_lo * P, np.int16),
                1: np.zeros(nt_hi * P, np.int16)}
        S = np.zeros((P, nt_tot, P), np.float32)
        for b in range(NB):
            for h in (0, 1):
                sl, doff = per_core[c].get(
                    (b, h), (np.zeros(0, np.int16), np.zeros(0, np.int64)))
                n = len(sl)
                for t in range(int(NT[b, h])):
                    _, p_, g = tile_pos[(b, h, t)]
                    seg_s = sl[t * P:(t + 1) * P]
                    seg_d = doff[t * P:(t + 1) * P]
                    flat[h][p_ * P:p_ * P + len(seg_s)] = seg_s
                    if len(seg_d):
                        S[rowsel[:len(seg_d)], g, seg_d] = 1.0
        base_c = c * SLICE
        invb = np.broadcast_to(inv_deg[base_c:base_c + SLICE],
                               (P, SLICE)).astype(ml_dtypes.bfloat16)
        core_data.append(dict(
            idx_lo=_wrap_idx(flat[0]) if nt_lo else np.zeros((P, 8), np.int16),
            idx_hi=_wrap_idx(flat[1]) if nt_hi else np.zeros((P, 8), np.int16),
            stab=S.reshape(P, nt_tot * P).astype(ml_dtypes.float8_e4m3),
            invb=np.ascontiguousarray(invb),
        ))

    # pooling one-hot [128, NB*NG] (pad nodes -> all-zero rows)
    ghots = []
    garange = np.arange(NG, dtype=np.float32)
    for c in range(NCORES):
        base = c * SLICE
        col = np.full(SLICE, -1.0, np.float32)
        npad = min(max(NN - base, 0), SLICE)
        if npad > 0:
            col[:npad] = batch_gid[base:base + npad]
        gid_pb = col.reshape(NB, P).T          # [P, NB]
        ghot = (gid_pb[:, :, None] == garange[None, None, :])
        ghots.append(np.ascontiguousarray(
            ghot.reshape(P, NB * NG)).astype(ml_dtypes.float8_e4m3))

    return dict(nt_lo=nt_lo, nt_hi=nt_hi, chunks=chunks,
                block_refs=block_refs, core_data=core_data, ghots=ghots)


def kernel(x, edge_index, u, batch, W_emb, b_emb, W_l, b_l, W_r, gamma, beta,
           W_g, b_g, W_f1, b_f1, W_f2, b_f2):
    x = np.asarray(x, np.float32)
    edge_index = np.asarray(edge_index)
    u = np.asarray(u, np.float32)
    batch = np.asarray(batch)

    src = edge_index[0].astype(np.int64)
    dst = edge_index[1].astype(np.int64)
    prep = _prepare(src, dst, batch.astype(np.float32))

    nt_lo, nt_hi = prep["nt_lo"], prep["nt_hi"]
    nt_tot = nt_lo + nt_hi
    chunks = prep["chunks"]
    block_refs = prep["block_refs"]

    xT = np.zeros((NODE_F, PADN), ml_dtypes.bfloat16)
    xT[:, :NN] = x.T.astype(ml_dtypes.bfloat16)

    inv_std = np.float32(1.0 / np.sqrt(1.0 + BN_EPS))
    gscale = np.asarray(gamma, np.float32) * inv_std  # [L, H]
    beta_np = np.asarray(beta, np.float32)

    ident_np = np.eye(P, dtype=ml_dtypes.bfloat16)
    ones_np = np.ones((P, 1), np.float32)

    # ------------------------------------------------------------------
    nc = bacc.Bacc(None, num_swdge_queues=4, dynamic_dma_scratch_size=32768)

    def din(name, shape, dtype=F32):
        return nc.dram_tensor(name, shape, dtype, kind="ExternalInput")

    xT_in = din("xT", [NODE_F, SLICE], BF16)
    idx_lo_in = din("idx_lo", [P, max(nt_lo, 1) * 8], I16)
    idx_hi_in = din("idx_hi", [P, max(nt_hi, 1) * 8], I16)
    stab_in = din("stab", [P, nt_tot * P], FP8)
    invb_in = din("invb", [P, SLICE], BF16)
    ghot_in = din("ghot", [P, NB * NG], FP8)
    ident_in = din("ident", [P, P], BF16)
    ones_in = din("ones", [P, 1])
    wemb_in = din("wemb", [NODE_F, H], BF16)
    bemb_in = din("bemb", [P, 1])
    wl_in = din("wl", [H, NL * H], BF16)
    wr_in = din("wr", [H, NL * H], BF16)
    gsbl_in = din("gsbl", [P, NL])
    gs_in = din("gs", [P, NL])
    bt_in = din("bt", [P, NL])
    uT_in = din("uT", [16, NG])
    wg_in = din("wg", [16, H])
    bg_in = din("bg", [P, 1])
    wf1_in = din("wf1", [2 * H, H])
    bf1_in = din("bf1", [P, 1])
    wf2_in = din("wf2", [H, 2])
    bf2_in = din("bf2", [2, 1])
    y_out = nc.dram_tensor("y", [2, NG], F32, kind="ExternalOutput")

    RG = [list(range(NCORES))]
    AluOp = mybir.AluOpType
    Act = mybir.ActivationFunctionType

    n_sch = -(-nt_tot // CH_TILES)   # S stream chunks
    sch_sizes = [min(CH_TILES, nt_tot - c0)
                 for c0 in range(0, nt_tot, CH_TILES)]

    with tile.TileContext(nc) as tc:
        with (
            tc.tile_pool(name="dram", bufs=1, space="DRAM") as dram,
            tc.tile_pool(name="meta", bufs=1) as meta,
            tc.tile_pool(name="hbuf", bufs=1) as hbuf,
            tc.tile_pool(name="glo", bufs=4) as glo,
            tc.tile_pool(name="ghi", bufs=4) as ghi,
            tc.tile_pool(name="oh", bufs=2) as ohp,
            tc.tile_pool(name="mean", bufs=3) as meanp,
            tc.tile_pool(name="pre", bufs=3) as prep_,
            tc.tile_pool(name="stg", bufs=3) as stgp,
            tc.tile_pool(name="ps_scat", bufs=2, space="PSUM") as ps_scat,
            tc.tile_pool(name="ps_dense", bufs=2, space="PSUM") as ps_dense,
            tc.tile_pool(name="ps_tr", bufs=2, space="PSUM") as ps_tr,
            tc.tile_pool(name="ps_pool", bufs=1, space="PSUM") as ps_pool,
            tc.tile_pool(name="ps_cnt", bufs=1, space="PSUM") as ps_cnt,
            tc.tile_pool(name="small", bufs=1) as small,
        ):
            # ---- constants & metadata
            idx_lo = meta.tile([P, max(nt_lo, 1) * 8], I16)
            idx_hi = meta.tile([P, max(nt_hi, 1) * 8], I16)
            stab_t = meta.tile([P, nt_tot * P], FP8)
            invb_t = meta.tile([P, SLICE], BF16)
            ghot_t = meta.tile([P, NB * NG], FP8)
            ident_t = meta.tile([P, P], BF16)
            ones_t = meta.tile([P, 1], F32)
            ones_bf = meta.tile([P, 1], BF16)
            xT_t = meta.tile([NODE_F, SLICE], BF16)
            wemb_t = meta.tile([NODE_F, H], BF16)
            bemb_t = meta.tile([P, 1], F32)
            wl_t = meta.tile([H, NL * H], BF16)
            wr_t = meta.tile([H, NL * H], BF16)
            gsbl_t = meta.tile([P, NL], F32)
            gs_t = meta.tile([P, NL], F32)
            bt_t = meta.tile([P, NL], F32)
            uT_t = meta.tile([16, NG], F32)
            wg_t = meta.tile([16, H], F32)
            bg_t = meta.tile([P, 1], F32)
            wf1a_t = meta.tile([H, H], F32)
            wf1b_t = meta.tile([H, H], F32)
            bf1_t = meta.tile([P, 1], F32)
            wf2_t = meta.tile([H, 2], F32)
            bf2_t = meta.tile([2, 1], F32)
            for t_, i_ in (
                (idx_lo, idx_lo_in), (idx_hi, idx_hi_in),
                (stab_t, stab_in), (invb_t, invb_in),
                (ghot_t, ghot_in),
                (ident_t, ident_in), (ones_t, ones_in), (xT_t, xT_in),
                (wemb_t, wemb_in), (bemb_t, bemb_in), (wl_t, wl_in),
                (wr_t, wr_in), (gsbl_t, gsbl_in), (gs_t, gs_in), (bt_t, bt_in),
                (uT_t, uT_in), (wg_t, wg_in), (bg_t, bg_in),
                (wf1a_t, wf1_in[:H, :]), (wf1b_t, wf1_in[H:, :]),
                (bf1_t, bf1_in), (wf2_t, wf2_in), (bf2_t, bf2_in),
            ):
                nc.sync.dma_start(t_[:], i_[:])

            nc.vector.tensor_copy(ones_bf[:], ones_t[:])
            # warm up the collective path so the embed AllGather doesn't pay
            # first-use setup on the critical path
            warm_in = dram.tile([16, 1], F32, name="warm_in")
            warm_out = dram.tile([NCORES * 16, 1], F32, name="warm_out",
                                 addr_space="Shared")
            nc.sync.dma_start(warm_in[:], ones_t[:16, :])
            nc.gpsimd.collective_compute(
                "AllGather", AluOp.bypass, replica_groups=RG,
                ins=[warm_in[:]], outs=[warm_out[:]],
            )
            hT_a = hbuf.tile([P, SLICE], BF16, name="hT_a")
            hT_b = hbuf.tile([P, SLICE], BF16, name="hT_b")

            slices = [dram.tile([SLICE, H], BF16, name=f"slice{i}") for i in range(NL)]
            tables = [dram.tile([PADN, H], BF16, addr_space="Shared", name=f"table{i}")
                      for i in range(NL)]
            payload = dram.tile([P + 1, NG], F32, name="payload")

            # ---- embed ----------------------------------------------------
            _sc_embed = nc.enter_named_scope("embed", False)
            for b in range(NB):
                hp = ps_dense.tile([P, P], F32, tag="d")
                nc.tensor.matmul(out=hp[:], lhsT=wemb_t[:],
                                 rhs=xT_t[:, b * P:(b + 1) * P],
                                 start=True, stop=True)
                nc.scalar.activation(hT_a[:, b * P:(b + 1) * P], hp[:],
                                     Act.Lrelu, bias=bemb_t[:], scale=1.0,
                                     alpha=NEG_SLOPE)
                tp = ps_tr.tile([P, P], BF16, tag="t")
                nc.tensor.transpose(out=tp[:], in_=hT_a[:, b * P:(b + 1) * P],
                                    identity=ident_t[:])
                stg = stgp.tile([P, P], BF16, tag="st")
                nc.vector.tensor_copy(stg[:], tp[:])
                nc.sync.dma_start(slices[0][b * P:(b + 1) * P, :], stg[:])

            nc.gpsimd.collective_compute(
                "AllGather", AluOp.bypass, replica_groups=RG,
                ins=[slices[0][:]], outs=[tables[0][:]],
            )
            nc.leave_named_scope("embed", _sc_embed[0], False)

            # ---- SAGE layers ---------------------------------------------
            hT_prev, hT_new = hT_a, hT_b
            pool_ps = ps_pool.tile([P, NG], F32, tag="pp")
            gcnt_ps = ps_cnt.tile([1, NG], F32, tag="c")

            for li in range(NL):
                _sc_l = nc.enter_named_scope(f"layer{li}", False)
                table_prev = tables[li]

                # gather preps + triggers (interleave lo/hi)
                sched = []
                for h, idx_t_, pool_h in ((0, idx_lo, glo), (1, idx_hi, ghi)):
                    c0 = 0
                    for ntc in chunks[h]:
                        sched.append((h, idx_t_, pool_h, ntc, c0))
                        c0 += ntc
                lo_s = [e for e in sched if e[0] == 0]
                hi_s = [e for e in sched if e[0] == 1]
                inter = []
                i = j = 0
                while i < len(lo_s) or j < len(hi_s):
                    if i < len(lo_s):
                        inter.append(lo_s[i]); i += 1
                    if j < len(hi_s):
                        inter.append(hi_s[j]); j += 1
                chunk_tiles = {0: [], 1: []}
                qrr = 0
                for h, idx_t_, pool_h, ntc, c0 in inter:
                    g = pool_h.tile([P, ntc, P], BF16, tag=f"g{h}")
                    nidx = ntc * P
                    fs = nidx // 16
                    f0 = c0 * P // 16
                    nc.gpsimd.dma_gather(
                        out_ap=g[:],
                        in_ap=table_prev[h * HALF:(h + 1) * HALF, :],
                        idxs_ap=idx_t_[:, f0:f0 + fs],
                        num_idxs=nidx, num_idxs_reg=nidx, elem_size=H,
                        queue_num=qrr % 4, single_packet=True,
                    )
                    qrr += 1
                    chunk_tiles[h].append(g)

                for b in range(NB):
                    sp = ps_scat.tile([P, P], F32, tag="sc")
                    refs = block_refs[b]
                    for i_r, (st, ch, slot, g) in enumerate(refs):
                        xs = chunk_tiles[st][ch][:, slot, :]
                        sref = stab_t[:, g * P:(g + 1) * P]
                        nc.tensor.matmul(out=sp[:], lhsT=xs, rhs=sref,
                                         start=(i_r == 0),
                                         stop=(i_r == len(refs) - 1))
                    mt = meanp.tile([P, P], BF16, tag="m")
                    nc.vector.tensor_tensor(
                        out=mt[:], in0=sp[:],
                        in1=invb_t[:, b * P:(b + 1) * P], op=AluOp.mult)
                    hp = ps_dense.tile([P, P], F32, tag="d")
                    nc.tensor.matmul(out=hp[:],
                                     lhsT=wl_t[:, li * H:(li + 1) * H],
                                     rhs=mt[:], start=True, stop=False)
                    nc.tensor.matmul(out=hp[:],
                                     lhsT=wr_t[:, li * H:(li + 1) * H],
                                     rhs=hT_prev[:, b * P:(b + 1) * P],
                                     start=False, stop=True)
                    pre = prep_.tile([P, P], F32, tag="p")
                    nc.scalar.activation(pre[:], hp[:], Act.Lrelu,
                                         bias=gsbl_t[:, li:li + 1],
                                         scale=gs_t[:, li:li + 1],
                                         alpha=NEG_SLOPE)
                    nc.vector.tensor_scalar(
                        out=hT_new[:, b * P:(b + 1) * P], in0=pre[:],
                        scalar1=bt_t[:, li:li + 1], scalar2=None,
                        op0=AluOp.add)
                    tp = ps_tr.tile([P, P], BF16, tag="t")
                    nc.tensor.transpose(out=tp[:],
                                        in_=hT_new[:, b * P:(b + 1) * P],
                                        identity=ident_t[:])
                    stg = stgp.tile([P, P], BF16, tag="st")
                    nc.vector.tensor_copy(stg[:], tp[:])
                    if li < NL - 1:
                        nc.sync.dma_start(slices[li + 1][b * P:(b + 1) * P, :],
                                          stg[:])
                    else:
                        gb = ghot_t[:, b * NG:(b + 1) * NG]
                        nc.tensor.matmul(out=pool_ps[:], lhsT=stg[:], rhs=gb,
                                         start=(b == 0), stop=(b == NB - 1))
                        nc.tensor.matmul(out=gcnt_ps[:], lhsT=ones_bf[:],
                                         rhs=gb,
                                         start=(b == 0), stop=(b == NB - 1))

                if li < NL - 1:
                    nc.gpsimd.collective_compute(
                        "AllGather", AluOp.bypass, replica_groups=RG,
                        ins=[slices[li + 1][:]], outs=[tables[li + 1][:]],
                    )
                hT_prev, hT_new = hT_new, hT_prev
                nc.leave_named_scope(f"layer{li}", _sc_l[0], False)

            # ---- pooling epilogue ----------------------------------------
            _sc_e = nc.enter_named_scope("epilogue", False)
            poolT = small.tile([P, NG], F32, tag="poolT")
            nc.vector.tensor_copy(poolT[:], pool_ps[:])
            gcrow = small.tile([1, NG], F32, tag="gcrow")
            nc.vector.tensor_copy(gcrow[:], gcnt_ps[:])
            nc.sync.dma_start(payload[:P, :], poolT[:])
            nc.sync.dma_start(payload[P:P + 1, :], gcrow[:])
            nc.gpsimd.collective_compute(
                "AllReduce", AluOp.add, replica_groups=RG,
                ins=[payload[:]], outs=[payload[:]],
            )
            pool_acc = small.tile([P, NG], F32, tag="pacc")
            gc_acc = small.tile([1, NG], F32, tag="gacc")
            nc.sync.dma_start(pool_acc[:], payload[:P, :])
            nc.sync.dma_start(gc_acc[:], payload[P:P + 1, :])
            nc.vector.tensor_scalar(out=gc_acc[:], in0=gc_acc[:], scalar1=1.0,
                                    scalar2=None, op0=AluOp.max)
            nc.vector.reciprocal(gc_acc[:], gc_acc[:])
            invg_row_d = dram.tile([1, NG], F32, name="invg_row_d")
            nc.sync.dma_start(invg_row_d[:], gc_acc[:])
            invg_bc = small.tile([P, NG], F32, tag="invgbc")
            nc.sync.dma_start(invg_bc[:],
                              invg_row_d[:1, :].to_broadcast((P, NG)))
            nc.vector.tensor_tensor(out=pool_acc[:], in0=pool_acc[:],
                                    in1=invg_bc[:], op=AluOp.mult)

            ug_ps = ps_dense.tile([P, NG], F32, tag="d")
            nc.tensor.matmul(out=ug_ps[:], lhsT=wg_t[:], rhs=uT_t[:],
                             start=True, stop=True)
            ugT = small.tile([P, NG], F32, tag="ugT")
            nc.scalar.activation(ugT[:], ug_ps[:], Act.Lrelu, bias=bg_t[:],
                                 scale=1.0, alpha=NEG_SLOPE)

            hid_ps = ps_dense.tile([P, NG], F32, tag="d")
            nc.tensor.matmul(out=hid_ps[:], lhsT=wf1a_t[:],
                             rhs=pool_acc[:], start=True, stop=False)
            nc.tensor.matmul(out=hid_ps[:], lhsT=wf1b_t[:], rhs=ugT[:],
                             start=False, stop=True)
            hidT = small.tile([P, NG], F32, tag="hidT")
            nc.scalar.activation(hidT[:], hid_ps[:], Act.Lrelu, bias=bf1_t[:],
                                 scale=1.0, alpha=NEG_SLOPE)

            y_ps = ps_tr.tile([2, NG], F32, tag="t")
            nc.tensor.matmul(out=y_ps[:], lhsT=wf2_t[:], rhs=hidT[:],
                             start=True, stop=True)
            yT = small.tile([2, NG], F32, tag="yT")
            nc.vector.tensor_scalar(out=yT[:], in0=y_ps[:], scalar1=bf2_t[:],
                                    scalar2=None, op0=AluOp.add)
            nc.sync.dma_start(y_out[:], yT[:])
            nc.leave_named_scope("epilogue", _sc_e[0], False)

    nc.finalize()
    _legalize_sync_waits(nc)

    common = dict(
        ident=ident_np, ones=ones_np,
        wemb=np.asarray(W_emb, np.float32).astype(ml_dtypes.bfloat16),
        bemb=np.asarray(b_emb, np.float32).reshape(P, 1),
        wl=np.asarray(W_l, np.float32).transpose(1, 0, 2).reshape(H, NL * H).astype(ml_dtypes.bfloat16),
        wr=np.asarray(W_r, np.float32).transpose(1, 0, 2).reshape(H, NL * H).astype(ml_dtypes.bfloat16),
        gsbl=(gscale * np.asarray(b_l, np.float32)).T.copy(),
        gs=gscale.T.copy(), bt=beta_np.T.copy(),
        uT=u.T.copy(),
        wg=np.asarray(W_g, np.float32),
        bg=np.asarray(b_g, np.float32).reshape(P, 1),
        wf1=np.asarray(W_f1, np.float32),
        bf1=np.asarray(b_f1, np.float32).reshape(P, 1),
        wf2=np.asarray(W_f2, np.float32),
        bf2=np.asarray(b_f2, np.float32).reshape(2, 1),
    )
    in_maps = []
    for c in range(NCORES):
        cd = prep["core_data"][c]
        in_maps.append(dict(
            common,
            xT=xT[:, c * SLICE:(c + 1) * SLICE].copy(),
            idx_lo=cd["idx_lo"], idx_hi=cd["idx_hi"],
            stab=cd["stab"], invb=cd["invb"], ghot=prep["ghots"][c],
        ))

    res = run_bass_kernel_spmd(nc, in_maps, core_ids=list(range(NCORES)),
                               trace=TRACE)
    global LAST_RESULT
    LAST_RESULT = res
    return np.asarray(res.results[0]["y"]).T.astype(np.float32).copy()


TRACE = False
LAST_RESULT = None
